# revision 2
# baseline (speedup 1.0000x reference)
"""Additive LoRA adapter (MoE-routed) forward — Trainium2, 8 NeuronCores.

Data-parallel over tokens: each core gets n/8 tokens, weights replicated.
Forward only => no collectives.

Per-core compute (feature-major / "transposed activations" layout):
  - base:   out.T[o,t] += sum_k Wt[k,o-block].T @ xbf[k,t]     (bf16 matmul)
  - router: h[hid,t] = silu(rw1t.T @ xf32 + rb1)               (fp32 matmul)
            logits[t,e] = (h_aug[.,t-block]).T @ rw2_aug       (fp32, bias+gates
            via the appended ones-row of h_aug)
  - top2 + softmax on VectorE via exp / two maxes / masks
  - coeff.T via PE transpose, expanded to (e,r)-rows via a 0/1 expand matmul
  - xa.T[(e,r),t] = At.T @ xbf ; wxa = xa * coeff_expand (DVE, -> bf16)
  - delta accumulated into the same PSUM group as base via Bf chunks
Host pre-transposes x/W/A/B so every matmul operand is a natural
(contraction-on-partitions) SBUF load; output is produced transposed and
un-transposed on the host.
"""
import sys

sys.path.insert(0, "/opt/trn_rl_repo")

import numpy as np
import ml_dtypes

from concourse import bacc, tile, mybir
from concourse.bass_utils import run_bass_kernel_spmd

N_CORES = 8
D = 2048          # d_in == d_out
E = 16            # populated experts
R = 16            # lora rank
ER = E * R        # 256
HID = 64          # router hidden
P = 128           # partitions
KC = D // P       # 16 contraction chunks
OC = D // P       # 16 output chunks
TT = 512          # token tile
ALPHA = 1.0
BACK_WARM_OC = 0  # first-tile output chunks padded with warm-up matmuls

F32 = mybir.dt.float32
BF16 = mybir.dt.bfloat16
AF = mybir.ActivationFunctionType
ALU = mybir.AluOpType
NEG_BIG = -1.0e30


def _build(n_core: int):
    NT = n_core // TT
    nc = bacc.Bacc("TRN2", target_bir_lowering=False, debug=False,
                   num_devices=N_CORES)

    NT_ = n_core // TT
    # x blocked per (tile, k-pair): [NT, KC/2, 128, 2, TT] -> 2KB contiguous
    # per partition per DMA
    xbf_d = nc.dram_tensor("xbf", [NT_, KC // 2, P, 2, TT], BF16,
                           kind="ExternalInput").ap()
    # W.T blocked i-major: [OC, 128i, KC, 128o] -> contiguous 4KB per
    # partition per oc-slab (DMA-efficient)
    wt_d = nc.dram_tensor("wt", [OC, P, KC, P], BF16, kind="ExternalInput").ap()
    at_d = nc.dram_tensor("at", [D, ER], BF16, kind="ExternalInput").ap()
    bf_d = nc.dram_tensor("bf", [ER, D], BF16, kind="ExternalInput").ap()
    rw1t_d = nc.dram_tensor("rw1t", [D, HID], BF16, kind="ExternalInput").ap()
    rb1_d = nc.dram_tensor("rb1", [HID, 1], F32, kind="ExternalInput").ap()
    rw2a_d = nc.dram_tensor("rw2a", [HID + 1, E], F32, kind="ExternalInput").ap()
    bias_d = nc.dram_tensor("biaspp", [P, OC], F32, kind="ExternalInput").ap()
    expand_d = nc.dram_tensor("expand", [E, ER], BF16, kind="ExternalInput").ap()
    ident_d = nc.dram_tensor("ident", [P, P], BF16, kind="ExternalInput").ap()
    outT_d = nc.dram_tensor("outT", [D, n_core], F32, kind="ExternalOutput").ap()

    with tile.TileContext(nc) as tc:
        with (
            tc.tile_pool(name="const", bufs=1) as constp,
            tc.tile_pool(name="wres", bufs=1) as wres,
            tc.tile_pool(name="xb", bufs=3) as xbp,
            tc.tile_pool(name="hp", bufs=2) as hp,
            tc.tile_pool(name="small", bufs=4) as smallp,
            tc.tile_pool(name="cf", bufs=2) as cfp,
            tc.tile_pool(name="outp", bufs=4) as outp,
            tc.tile_pool(name="ps_out", bufs=2, space="PSUM") as ps_out,
            tc.tile_pool(name="ps_warm", bufs=1, space="PSUM") as ps_warm,
            tc.tile_pool(name="ps_xa", bufs=1, space="PSUM") as ps_xa,
            tc.tile_pool(name="ps_h", bufs=1, space="PSUM") as ps_h,
            tc.tile_pool(name="ps_sm", bufs=1, space="PSUM") as ps_sm,
        ):
            # ---- small constants ----
            rw1t_sb = constp.tile([P, KC, HID], BF16)
            for k in range(KC):
                nc.sync.dma_start(out=rw1t_sb[:, k, :],
                                  in_=rw1t_d[k * P:(k + 1) * P, :])
            rb1_sb = constp.tile([HID, 1], F32)
            nc.sync.dma_start(out=rb1_sb[:], in_=rb1_d[:])
            rw2a_sb = constp.tile([HID + 1, E], F32)
            nc.sync.dma_start(out=rw2a_sb[:], in_=rw2a_d[:])
            bias_sb = constp.tile([P, OC], F32)
            nc.sync.dma_start(out=bias_sb[:], in_=bias_d[:])
            expand_sb = constp.tile([E, ER], BF16)
            nc.sync.dma_start(out=expand_sb[:], in_=expand_d[:])
            ident_sb = constp.tile([P, P], BF16)
            nc.sync.dma_start(out=ident_sb[:], in_=ident_d[:])

            def load_x_tile(tt, at_sb=None):
                # interleave x chunk-pairs with A.T chunks (tile 0 only) so
                # the router and xa inputs stream in together at startup
                xb_sb = xbp.tile([P, KC, TT], BF16)
                for j in range(KC // 2):
                    nc.sync.dma_start(out=xb_sb[:, 2 * j:2 * j + 2, :],
                                      in_=xbf_d[tt, j, :, :, :])
                    if at_sb is not None:
                        for c in range(2):
                            k = 2 * j + c
                            nc.sync.dma_start(out=at_sb[:, k, :],
                                              in_=at_d[k * P:(k + 1) * P, :])
                return xb_sb

            at_sb = wres.tile([P, KC, ER], BF16)
            x_tile0 = load_x_tile(0, at_sb=at_sb)

            # ---- resident weights; tile-1 x hoisted ahead of W so the
            # tile boundary never starves ----
            x_tile1 = load_x_tile(1) if NT > 1 else None
            bf_sb = wres.tile([P, 2, D], BF16)
            for k2 in range(2):
                nc.sync.dma_start(out=bf_sb[:, k2, :],
                                  in_=bf_d[k2 * P:(k2 + 1) * P, :])
            x_tile2 = None
            wt_sb = wres.tile([P, OC, KC, P], BF16)
            for oc in range(OC):
                nc.sync.dma_start(
                    out=wt_sb[:, oc, :, :],
                    in_=wt_d[oc, :, :, :])
                if oc == 9 and NT > 2:
                    # slot tile-2's x into the W stream: lands before the
                    # pipeline needs it, without delaying early W chunks
                    x_tile2 = load_x_tile(2)

            warm = [None, None]

            def front(tt):
                # router + xa + top2 + wxa: everything that does NOT need W.
                # Emitted one tile ahead of back() so the PE always has
                # W-independent work while wt/xb stream in.
                t0 = tt * TT
                if tt == 0:
                    xb_sb = x_tile0
                elif tt == 1:
                    xb_sb = x_tile1
                elif tt == 2:
                    xb_sb = x_tile2
                else:
                    xb_sb = load_x_tile(tt)

                # ---- router hidden: h = silu(rw1t.T @ x + rb1) (bf16 mm) ----
                h_ps = ps_h.tile([HID, TT], F32)
                if tt == 0:
                    # dummy matmuls on a memset scratch tile (no DMA deps):
                    # fill the ~22us DMA spin-up so the PE is busy and HAM
                    # stays at 2.4GHz when real work arrives
                    scr_sb = constp.tile([P, TT], BF16)
                    nc.vector.memset(scr_sb[:], 1.0)
                    warm_ps = ps_warm.tile([P, TT], F32)
                    warm[0], warm[1] = warm_ps, scr_sb
                    for _ in range(84):
                        nc.tensor.matmul(warm_ps[:], lhsT=scr_sb[:, 0:P],
                                         rhs=scr_sb[:], start=True,
                                         stop=True)
                for k in range(KC):
                    nc.tensor.matmul(h_ps[:], lhsT=rw1t_sb[:, k, :],
                                     rhs=xb_sb[:, k, :],
                                     start=(k == 0), stop=(k == KC - 1))
                h_sb = hp.tile([HID + 1, TT], F32)
                nc.vector.memset(h_sb[HID:HID + 1, :], 1.0)

                # ---- silu + logits per 128-token sub-chunk (fp32 mm) ----
                lg_ps = ps_sm.tile([P, TT // P, E], F32, tag="lgct")
                for s in range(TT // P):
                    nc.scalar.activation(h_sb[0:HID, s * P:(s + 1) * P],
                                         h_ps[:, s * P:(s + 1) * P], AF.Silu,
                                         bias=rb1_sb[:], scale=1.0)
                    nc.tensor.matmul(
                        lg_ps[:, s, :], lhsT=h_sb[:, s * P:(s + 1) * P],
                        rhs=rw2a_sb[:], start=True, stop=True)

                # ---- xa = At.T @ xbf (bf16), two 128-row halves ----
                xa_ps = []
                for half in range(2):
                    xp = ps_xa.tile([P, TT], F32, tag=f"xa{half}")
                    for k in range(KC):
                        nc.tensor.matmul(
                            xp[:], lhsT=at_sb[:, k, half * P:(half + 1) * P],
                            rhs=xb_sb[:, k, :],
                            start=(k == 0), stop=(k == KC - 1))
                    xa_ps.append(xp)

                # ---- top2 + softmax -> coeff (token-major), on DVE/ACT ----
                coeff_bfs = []
                for s in range(TT // P):
                    e_sb = smallp.tile([P, E], F32, tag=f"e{s % 2}")
                    nc.scalar.activation(e_sb[:], lg_ps[:, s, :], AF.Exp)
                    m1 = smallp.tile([P, 1], F32, tag="m1")
                    nc.vector.tensor_reduce(m1[:], e_sb[:],
                                            axis=mybir.AxisListType.X,
                                            op=ALU.max)
                    mask1 = smallp.tile([P, E], F32, tag="mask1")
                    nc.vector.tensor_scalar(mask1[:], e_sb[:], m1[:], None,
                                            op0=ALU.is_ge)
                    masked = smallp.tile([P, E], F32, tag="masked")
                    nc.vector.scalar_tensor_tensor(
                        masked[:], in0=mask1[:], scalar=NEG_BIG, in1=e_sb[:],
                        op0=ALU.mult, op1=ALU.add)
                    m2 = smallp.tile([P, 1], F32, tag="m2")
                    nc.vector.tensor_reduce(m2[:], masked[:],
                                            axis=mybir.AxisListType.X,
                                            op=ALU.max)
                    s12 = smallp.tile([P, 1], F32, tag="s12")
                    nc.vector.tensor_tensor(s12[:], m1[:], m2[:], op=ALU.add)
                    rs = smallp.tile([P, 1], F32, tag="rs")
                    nc.vector.reciprocal(rs[:], s12[:])
                    mask2 = smallp.tile([P, E], F32, tag="mask2")
                    nc.vector.tensor_scalar(mask2[:], e_sb[:], m2[:], None,
                                            op0=ALU.is_ge)
                    coeff_bf = smallp.tile([P, E], BF16, tag=f"coeff{s % 2}")
                    nc.vector.scalar_tensor_tensor(
                        coeff_bf[:], in0=e_sb[:], scalar=rs[:], in1=mask2[:],
                        op0=ALU.mult, op1=ALU.mult)
                    coeff_bfs.append(coeff_bf)

                # ---- PE transpose coeff [128,16] -> [16,128] x4, one bank ----
                ct_ps = ps_sm.tile([E, TT // P, P], BF16, tag="lgct")
                for s in range(TT // P):
                    nc.tensor.transpose(ct_ps[:, s, :], coeff_bfs[s][:],
                                        ident_sb[:])
                coefft_sb = cfp.tile([E, TT], BF16)
                nc.vector.tensor_copy(coefft_sb[:], ct_ps[:])

                # ---- expand coeff.T rows to (e,r) rows; wxa = xa * cexp ----
                wxa_sb = cfp.tile([P, 2, TT], BF16, tag="wxa")
                for half in range(2):
                    cx_ps = ps_sm.tile([P, TT], F32, tag="cx")
                    nc.tensor.matmul(
                        cx_ps[:], lhsT=expand_sb[:, half * P:(half + 1) * P],
                        rhs=coefft_sb[:], start=True, stop=True)
                    cx_sb = cfp.tile([P, TT], F32, tag=f"cxs{half}")
                    nc.vector.tensor_copy(cx_sb[:], cx_ps[:])
                    nc.vector.tensor_tensor(wxa_sb[:, half, :], xa_ps[half][:],
                                            cx_sb[:], op=ALU.mult)
                return xb_sb, wxa_sb

            def back(tt, state):
                # base + delta accumulated per 128-row output chunk
                t0 = tt * TT
                xb_sb, wxa_sb = state
                for oc in range(OC):
                    if tt == 0 and oc < BACK_WARM_OC and warm[0] is not None:
                        # cover W-chunk arrival jitter in the first tile so
                        # the PE never idles long enough for HAM to rethrottle
                        for _ in range(3):
                            nc.tensor.matmul(warm[0][:], lhsT=warm[1][:, 0:P],
                                             rhs=warm[1][:], start=True,
                                             stop=True)
                    ps = ps_out.tile([P, TT], F32)
                    for k in range(KC):
                        nc.tensor.matmul(ps[:], lhsT=wt_sb[:, oc, k, :],
                                         rhs=xb_sb[:, k, :],
                                         start=(k == 0), stop=False)
                    for k2 in range(2):
                        nc.tensor.matmul(ps[:], lhsT=bf_sb[:, k2, oc * P:(oc + 1) * P],
                                         rhs=wxa_sb[:, k2, :],
                                         start=False, stop=(k2 == 1))
                    o_sb = outp.tile([P, TT], F32)
                    # epilogue on ACT: keeps the PSUM drain off DVE, which
                    # is busy with the next tile's top-2 chain
                    nc.scalar.activation(o_sb[:], ps[:], AF.Identity,
                                         bias=bias_sb[:, oc:oc + 1],
                                         scale=1.0)
                    nc.sync.dma_start(
                        out=outT_d[oc * P:(oc + 1) * P, t0:t0 + TT],
                        in_=o_sb[:])

            # one-tile-deep software pipeline: front(j+1) fills the PE
            # while back(j) waits on wt / PSUM drains
            states = {0: front(0)}
            for tt in range(NT):
                if tt + 1 < NT:
                    states[tt + 1] = front(tt + 1)
                back(tt, states.pop(tt))

    nc.compile()
    return nc


_CACHE = {}


def _get_nc(n_core: int):
    if n_core not in _CACHE:
        _CACHE[n_core] = _build(n_core)
    return _CACHE[n_core]


def _prep_in_maps(x, W, bias, rw1, rb1, rw2, rb2, A, B, gates):
    x, W, bias, rw1, rb1, rw2, rb2, A, B, gates = (
        np.asarray(v) for v in (x, W, bias, rw1, rb1, rw2, rb2, A, B, gates))
    xf = np.ascontiguousarray(x.reshape(-1, D).astype(np.float32))
    n = xf.shape[0]
    assert n % N_CORES == 0
    n_core = n // N_CORES

    bf16 = ml_dtypes.bfloat16
    xTb = np.ascontiguousarray(xf.T).astype(bf16)        # [D, n] bf16
    # W.T blocked i-major [OC, 128i, KC, 128o]
    wt = np.ascontiguousarray(
        W.astype(np.float32).T.reshape(KC, P, OC, P).transpose(2, 1, 0, 3)
    ).astype(bf16)
    at = np.ascontiguousarray(
        A.astype(np.float32).reshape(ER, D).T).astype(bf16)
    bfl = np.ascontiguousarray(
        B.astype(np.float32).transpose(0, 2, 1).reshape(ER, D)).astype(bf16)
    rw1t = np.ascontiguousarray(rw1.astype(np.float32).T).astype(bf16)
    rb1c = np.ascontiguousarray(rb1.astype(np.float32).reshape(HID, 1))
    rw2a = np.concatenate(
        [rw2[:E].astype(np.float32).T,
         (rb2[:E].astype(np.float32) + gates.astype(np.float32))[None, :]],
        axis=0)
    rw2a = np.ascontiguousarray(rw2a)
    biaspp = np.ascontiguousarray(
        bias.astype(np.float32).reshape(OC, P).T)
    expand = np.zeros((E, ER), np.float32)
    for e in range(E):
        expand[e, e * R:(e + 1) * R] = ALPHA
    expand = expand.astype(bf16)
    ident = np.eye(P, dtype=np.float32).astype(bf16)

    shared = {"wt": wt, "at": at, "bf": bfl, "rw1t": rw1t, "rb1": rb1c,
              "rw2a": rw2a, "biaspp": biaspp, "expand": expand, "ident": ident}
    NT = n_core // TT
    in_maps = []
    for c in range(N_CORES):
        sl = slice(c * n_core, (c + 1) * n_core)
        xc = (xTb[:, sl].reshape(KC // 2, 2, P, NT, TT)
              .transpose(3, 0, 2, 1, 4))
        in_maps.append({"xbf": np.ascontiguousarray(xc), **shared})
    return in_maps, n_core


def _core_out(result_map):
    # per-core unshard: kernel emits the output transposed [D, n_core]
    return result_map["outT"].T


def kernel(x, W, bias, rw1, rb1, rw2, rb2, A, B, gates):
    lead = x.shape[:-1]
    in_maps, n_core = _prep_in_maps(x, W, bias, rw1, rb1, rw2, rb2, A, B,
                                    gates)
    n = n_core * N_CORES
    nc = _get_nc(n_core)
    res = None
    for attempt in range(3):
        try:
            res = run_bass_kernel_spmd(nc, in_maps,
                                       core_ids=list(range(N_CORES)))
            break
        except Exception:
            # sporadic NRT_EXEC_UNIT_UNRECOVERABLE on a fresh NEFF; retry
            if attempt == 2:
                raise
            import time as _time

            _time.sleep(10)

    out = np.empty((n, D), np.float32)
    for c in range(N_CORES):
        out[c * n_core:(c + 1) * n_core] = res.results[c]["outT"].T
    return out.reshape(*lead, D)



# revision 6
# speedup vs baseline: 1.0215x; 1.0215x over previous
"""Additive LoRA adapter (MoE-routed) forward — Trainium2, 8 NeuronCores.

Data-parallel over tokens: each core gets n/8 tokens, weights replicated.
Forward only => no collectives.

Per-core compute (feature-major / "transposed activations" layout):
  - base:   out.T[o,t] += sum_k Wt[k,o-block].T @ xbf[k,t]     (bf16 matmul)
  - router: h[hid,t] = silu(rw1t.T @ xf32 + rb1)               (bf16 matmul)
            logits[t,e] = (h_aug[.,t-block]).T @ rw2_aug       (bf16, bias+gates
            via the appended ones-row of h_aug)
  - top2 + softmax on VectorE via exp / two maxes / masks
  - coeff.T via PE transpose, expanded to (e,r)-rows via a 0/1 expand matmul
  - xa.T[(e,r),t] = At.T @ xbf ; wxa = xa * coeff_expand (DVE, -> bf16)
  - delta accumulated into the same PSUM group as base via Bf chunks

Pipeline: front(t+1) (router/xa/top2/wxa — no W needed) is interleaved
into back(t)'s output-chunk loop so the PE never outruns the x DMA
stream at tile boundaries, and the DVE coeff chain always has matmul
work running beside it. Output stored transposed in bf16 and
un-transposed/upcast on the host.
"""
import sys

sys.path.insert(0, "/opt/trn_rl_repo")

import numpy as np
import ml_dtypes

from concourse import bacc, tile, mybir
from concourse.bass_utils import run_bass_kernel_spmd

N_CORES = 8
D = 2048          # d_in == d_out
E = 16            # populated experts
R = 16            # lora rank
ER = E * R        # 256
HID = 64          # router hidden
P = 128           # partitions
KC = D // P       # 16 contraction chunks
OC = D // P       # 16 output chunks
TT = 512          # token tile
ALPHA = 1.0
WARM = 28         # startup dummy matmuls (cover DMA spin-up, warm HAM)

F32 = mybir.dt.float32
BF16 = mybir.dt.bfloat16
AF = mybir.ActivationFunctionType
ALU = mybir.AluOpType
NEG_BIG = -1.0e30


def _build(n_core: int):
    NT = n_core // TT
    nc = bacc.Bacc("TRN2", target_bir_lowering=False, debug=False,
                   num_devices=N_CORES)

    # x blocked per (tile, k-pair): [NT, KC/2, 128, 2, TT] -> 2KB contiguous
    # per partition per DMA
    xbf_d = nc.dram_tensor("xbf", [NT, KC // 2, P, 2, TT], BF16,
                           kind="ExternalInput").ap()
    # W.T blocked i-major: [OC, 128i, KC, 128o] -> contiguous 4KB per
    # partition per oc-slab (DMA-efficient)
    wt_d = nc.dram_tensor("wt", [OC, P, KC, P], BF16, kind="ExternalInput").ap()
    at_d = nc.dram_tensor("at", [D, ER], BF16, kind="ExternalInput").ap()
    bf_d = nc.dram_tensor("bf", [ER, D], BF16, kind="ExternalInput").ap()
    rw1t_d = nc.dram_tensor("rw1t", [D, HID], BF16, kind="ExternalInput").ap()
    rb1_d = nc.dram_tensor("rb1", [HID, 1], F32, kind="ExternalInput").ap()
    rw2a_d = nc.dram_tensor("rw2a", [HID + 1, E], BF16, kind="ExternalInput").ap()
    bias_d = nc.dram_tensor("biaspp", [P, OC], F32, kind="ExternalInput").ap()
    expand_d = nc.dram_tensor("expand", [E, ER], BF16, kind="ExternalInput").ap()
    ident_d = nc.dram_tensor("ident", [P, P], BF16, kind="ExternalInput").ap()
    outT_d = nc.dram_tensor("outT", [D, n_core], BF16, kind="ExternalOutput").ap()

    with tile.TileContext(nc) as tc:
        with (
            tc.tile_pool(name="const", bufs=1) as constp,
            tc.tile_pool(name="wres", bufs=1) as wres,
            tc.tile_pool(name="xb", bufs=3) as xbp,
            tc.tile_pool(name="hp", bufs=2) as hp,
            tc.tile_pool(name="small", bufs=4) as smallp,
            tc.tile_pool(name="cf", bufs=2) as cfp,
            tc.tile_pool(name="outp", bufs=4) as outp,
            tc.tile_pool(name="ps_out", bufs=3, space="PSUM") as ps_out,
            tc.tile_pool(name="ps_xa", bufs=1, space="PSUM") as ps_xa,
            tc.tile_pool(name="ps_h", bufs=1, space="PSUM") as ps_h,
            tc.tile_pool(name="ps_sm", bufs=1, space="PSUM") as ps_sm,
        ):
            # ---- small constants ----
            rw1t_sb = constp.tile([P, KC, HID], BF16)
            for k in range(KC):
                nc.sync.dma_start(out=rw1t_sb[:, k, :],
                                  in_=rw1t_d[k * P:(k + 1) * P, :])
            rb1_sb = constp.tile([HID, 1], F32)
            nc.sync.dma_start(out=rb1_sb[:], in_=rb1_d[:])
            rw2a_sb = constp.tile([HID + 1, E], BF16)
            nc.sync.dma_start(out=rw2a_sb[:], in_=rw2a_d[:])
            bias_sb = constp.tile([P, OC], F32)
            nc.sync.dma_start(out=bias_sb[:], in_=bias_d[:])
            expand_sb = constp.tile([E, ER], BF16)
            nc.sync.dma_start(out=expand_sb[:], in_=expand_d[:])
            ident_sb = constp.tile([P, P], BF16)
            nc.sync.dma_start(out=ident_sb[:], in_=ident_d[:])

            x_tiles = {}

            def load_x_tile(tt, at_sb=None):
                # interleave x chunk-pairs with A.T chunks (tile 0 only) so
                # the router and xa inputs stream in together at startup
                xb_sb = xbp.tile([P, KC, TT], BF16, tag="xb")
                for j in range(KC // 2):
                    nc.sync.dma_start(out=xb_sb[:, 2 * j:2 * j + 2, :],
                                      in_=xbf_d[tt, j, :, :, :])
                    if at_sb is not None:
                        for c in range(2):
                            k = 2 * j + c
                            nc.sync.dma_start(out=at_sb[:, k, :],
                                              in_=at_d[k * P:(k + 1) * P, :])
                x_tiles[tt] = xb_sb

            at_sb = wres.tile([P, KC, ER], BF16)
            load_x_tile(0, at_sb=at_sb)
            # tile-1 x hoisted ahead of W so the first tile boundary never
            # starves; later tiles stream during back() (2 tiles ahead)
            if NT > 1:
                load_x_tile(1)
            bf_sb = wres.tile([P, 2, D], BF16)
            for k2 in range(2):
                nc.sync.dma_start(out=bf_sb[:, k2, :],
                                  in_=bf_d[k2 * P:(k2 + 1) * P, :])
            wt_sb = wres.tile([P, OC, KC, P], BF16)
            for oc in range(OC):
                nc.sync.dma_start(out=wt_sb[:, oc, :, :],
                                  in_=wt_d[oc, :, :, :])

            # ---- startup dummy matmuls: cover the DMA spin-up so HAM
            # reaches 2.4GHz before real work arrives ----
            scr_sb = constp.tile([P, TT], BF16)
            nc.vector.memset(scr_sb[:], 1.0)
            warm_ps = ps_out.tile([P, TT], F32, tag="ps", name="ps")
            for _ in range(WARM):
                nc.tensor.matmul(warm_ps[:], lhsT=scr_sb[:, 0:P],
                                 rhs=scr_sb[:], start=True, stop=True)

            def make_front(tt):
                # router + xa + top2 + wxa for tile tt: everything that does
                # NOT need W, split into pieces interleaved into back(tt-1)
                st = {"xb": x_tiles[tt]}
                xb_sb = st["xb"]

                def p_router():
                    h_ps = ps_h.tile([HID, TT], F32, tag="h")
                    for k in range(KC):
                        nc.tensor.matmul(h_ps[:], lhsT=rw1t_sb[:, k, :],
                                         rhs=xb_sb[:, k, :],
                                         start=(k == 0), stop=(k == KC - 1))
                    h_sb = hp.tile([HID + 1, TT], BF16)
                    nc.vector.memset(h_sb[HID:HID + 1, :], 1.0)
                    lg_ps = ps_sm.tile([P, TT // P, E], F32, tag="lgct")
                    for s in range(TT // P):
                        nc.scalar.activation(h_sb[0:HID, s * P:(s + 1) * P],
                                             h_ps[:, s * P:(s + 1) * P],
                                             AF.Silu, bias=rb1_sb[:], scale=1.0)
                        nc.tensor.matmul(
                            lg_ps[:, s, :], lhsT=h_sb[:, s * P:(s + 1) * P],
                            rhs=rw2a_sb[:], start=True, stop=True)
                    st["lg"] = lg_ps

                def p_xa(half):
                    xp = ps_xa.tile([P, TT], F32, tag=f"xa{half}",
                                    name=f"xa{half}")
                    for k in range(KC):
                        nc.tensor.matmul(
                            xp[:], lhsT=at_sb[:, k, half * P:(half + 1) * P],
                            rhs=xb_sb[:, k, :],
                            start=(k == 0), stop=(k == KC - 1))
                    st[f"xa{half}"] = xp

                def p_top2():
                    # top2 + softmax -> coeff (token-major), on DVE/ACT
                    lg_ps = st["lg"]
                    coeff_bfs = []
                    for s in range(TT // P):
                        e_sb = smallp.tile([P, E], F32, tag=f"e{s % 2}",
                                           name="e_sb")
                        nc.scalar.activation(e_sb[:], lg_ps[:, s, :], AF.Exp)
                        m1 = smallp.tile([P, 1], F32, tag="m1")
                        nc.vector.tensor_reduce(m1[:], e_sb[:],
                                                axis=mybir.AxisListType.X,
                                                op=ALU.max)
                        mask1 = smallp.tile([P, E], F32, tag="mask1")
                        nc.vector.tensor_scalar(mask1[:], e_sb[:], m1[:], None,
                                                op0=ALU.is_ge)
                        masked = smallp.tile([P, E], F32, tag="masked")
                        nc.vector.scalar_tensor_tensor(
                            masked[:], in0=mask1[:], scalar=NEG_BIG,
                            in1=e_sb[:], op0=ALU.mult, op1=ALU.add)
                        m2 = smallp.tile([P, 1], F32, tag="m2")
                        nc.vector.tensor_reduce(m2[:], masked[:],
                                                axis=mybir.AxisListType.X,
                                                op=ALU.max)
                        s12 = smallp.tile([P, 1], F32, tag="s12")
                        nc.vector.tensor_tensor(s12[:], m1[:], m2[:],
                                                op=ALU.add)
                        rs = smallp.tile([P, 1], F32, tag="rs")
                        nc.vector.reciprocal(rs[:], s12[:])
                        mask2 = smallp.tile([P, E], F32, tag="mask2")
                        nc.vector.tensor_scalar(mask2[:], e_sb[:], m2[:], None,
                                                op0=ALU.is_ge)
                        coeff_bf = smallp.tile([P, E], BF16,
                                               tag=f"coeff{s % 2}",
                                               name="coeff_bf")
                        nc.vector.scalar_tensor_tensor(
                            coeff_bf[:], in0=e_sb[:], scalar=rs[:],
                            in1=mask2[:], op0=ALU.mult, op1=ALU.mult)
                        coeff_bfs.append(coeff_bf)
                    st["coeffs"] = coeff_bfs

                def p_transp():
                    # PE transpose coeff [128,16] -> [16,128] x4, one bank
                    ct_ps = ps_sm.tile([E, TT // P, P], BF16, tag="lgct",
                                       name="ct_ps")
                    for s in range(TT // P):
                        nc.tensor.transpose(ct_ps[:, s, :], st["coeffs"][s][:],
                                            ident_sb[:])
                    coefft_sb = cfp.tile([E, TT], BF16, tag="coefft")
                    nc.vector.tensor_copy(coefft_sb[:], ct_ps[:])
                    st["coefft"] = coefft_sb

                def p_wxa():
                    # expand coeff.T rows to (e,r) rows; wxa = xa * cexp
                    wxa_sb = cfp.tile([P, 2, TT], BF16, tag="wxa")
                    for half in range(2):
                        cx_ps = ps_sm.tile([P, TT], F32, tag="cx",
                                           name="cx_ps")
                        nc.tensor.matmul(
                            cx_ps[:],
                            lhsT=expand_sb[:, half * P:(half + 1) * P],
                            rhs=st["coefft"][:], start=True, stop=True)
                        cx_sb = cfp.tile([P, TT], F32, tag=f"cxs{half}",
                                         name="cx_sb")
                        nc.vector.tensor_copy(cx_sb[:], cx_ps[:])
                        nc.vector.tensor_tensor(wxa_sb[:, half, :],
                                                st[f"xa{half}"][:],
                                                cx_sb[:], op=ALU.mult)
                    st["wxa"] = wxa_sb

                pieces = [p_router, lambda: p_xa(0), lambda: p_xa(1),
                          p_top2, p_transp, p_wxa]
                return st, pieces

            def back_oc(tt, st, oc):
                # base + delta accumulated for one 128-row output chunk
                t0 = tt * TT
                xb_sb, wxa_sb = st["xb"], st["wxa"]
                ps = ps_out.tile([P, TT], F32)
                for k in range(KC):
                    nc.tensor.matmul(ps[:], lhsT=wt_sb[:, oc, k, :],
                                     rhs=xb_sb[:, k, :],
                                     start=(k == 0), stop=False)
                for k2 in range(2):
                    nc.tensor.matmul(ps[:],
                                     lhsT=bf_sb[:, k2, oc * P:(oc + 1) * P],
                                     rhs=wxa_sb[:, k2, :],
                                     start=False, stop=(k2 == 1))
                o_sb = outp.tile([P, TT], BF16)
                # epilogue (bias add + bf16 cast) on DVE: no ACT table
                # thrash, ACT keeps only silu/exp
                nc.vector.tensor_scalar(o_sb[:], ps[:], bias_sb[:, oc:oc + 1],
                                        None, op0=ALU.add)
                nc.sync.dma_start(
                    out=outT_d[oc * P:(oc + 1) * P, t0:t0 + TT],
                    in_=o_sb[:])

            # front(t+1) pieces slotted into back(t)'s oc loop: PE always
            # has W-independent work while wt/xb stream in, and the DVE
            # coeff chain for t+1 runs beside back(t)'s matmuls.
            SLOT = {1: 0, 3: 1, 5: 2, 7: 3, 9: 4, 12: 5}
            st0, pieces0 = make_front(0)
            for p in pieces0:
                p()
            states = {0: st0}
            for tt in range(NT):
                st = states.pop(tt)
                if tt + 1 < NT:
                    nst, pieces = make_front(tt + 1)
                    states[tt + 1] = nst
                else:
                    pieces = []
                for oc in range(OC):
                    back_oc(tt, st, oc)
                    if oc == 0 and tt + 2 < NT:
                        load_x_tile(tt + 2)
                    if oc in SLOT and pieces:
                        pieces[SLOT[oc]]()

    nc.compile()
    return nc


_CACHE = {}


def _get_nc(n_core: int):
    if n_core not in _CACHE:
        _CACHE[n_core] = _build(n_core)
    return _CACHE[n_core]


def _prep_in_maps(x, W, bias, rw1, rb1, rw2, rb2, A, B, gates):
    x, W, bias, rw1, rb1, rw2, rb2, A, B, gates = (
        np.asarray(v) for v in (x, W, bias, rw1, rb1, rw2, rb2, A, B, gates))
    xf = np.ascontiguousarray(x.reshape(-1, D).astype(np.float32))
    n = xf.shape[0]
    assert n % N_CORES == 0
    n_core = n // N_CORES

    bf16 = ml_dtypes.bfloat16
    xTb = np.ascontiguousarray(xf.T).astype(bf16)        # [D, n] bf16
    # W.T blocked i-major [OC, 128i, KC, 128o]
    wt = np.ascontiguousarray(
        W.astype(np.float32).T.reshape(KC, P, OC, P).transpose(2, 1, 0, 3)
    ).astype(bf16)
    at = np.ascontiguousarray(
        A.astype(np.float32).reshape(ER, D).T).astype(bf16)
    bfl = np.ascontiguousarray(
        B.astype(np.float32).transpose(0, 2, 1).reshape(ER, D)).astype(bf16)
    rw1t = np.ascontiguousarray(rw1.astype(np.float32).T).astype(bf16)
    rb1c = np.ascontiguousarray(rb1.astype(np.float32).reshape(HID, 1))
    rw2a = np.concatenate(
        [rw2[:E].astype(np.float32).T,
         (rb2[:E].astype(np.float32) + gates.astype(np.float32))[None, :]],
        axis=0)
    rw2a = np.ascontiguousarray(rw2a).astype(bf16)
    biaspp = np.ascontiguousarray(
        bias.astype(np.float32).reshape(OC, P).T)
    expand = np.zeros((E, ER), np.float32)
    for e in range(E):
        expand[e, e * R:(e + 1) * R] = ALPHA
    expand = expand.astype(bf16)
    ident = np.eye(P, dtype=np.float32).astype(bf16)

    shared = {"wt": wt, "at": at, "bf": bfl, "rw1t": rw1t, "rb1": rb1c,
              "rw2a": rw2a, "biaspp": biaspp, "expand": expand, "ident": ident}
    NT = n_core // TT
    in_maps = []
    for c in range(N_CORES):
        sl = slice(c * n_core, (c + 1) * n_core)
        xc = (xTb[:, sl].reshape(KC // 2, 2, P, NT, TT)
              .transpose(3, 0, 2, 1, 4))
        in_maps.append({"xbf": np.ascontiguousarray(xc), **shared})
    return in_maps, n_core


def _core_out(result_map):
    # per-core unshard: kernel emits the output transposed bf16 [D, n_core]
    return np.asarray(result_map["outT"]).astype(np.float32).T


def kernel(x, W, bias, rw1, rb1, rw2, rb2, A, B, gates):
    lead = x.shape[:-1]
    in_maps, n_core = _prep_in_maps(x, W, bias, rw1, rb1, rw2, rb2, A, B,
                                    gates)
    n = n_core * N_CORES
    nc = _get_nc(n_core)
    res = None
    for attempt in range(3):
        try:
            res = run_bass_kernel_spmd(nc, in_maps,
                                       core_ids=list(range(N_CORES)))
            break
        except Exception:
            # sporadic NRT_EXEC_UNIT_UNRECOVERABLE on a fresh NEFF; retry
            if attempt == 2:
                raise
            import time as _time

            _time.sleep(10)

    out = np.empty((n, D), np.float32)
    for c in range(N_CORES):
        out[c * n_core:(c + 1) * n_core] = _core_out(res.results[c])
    return out.reshape(*lead, D)
